# revision 7
# baseline (speedup 1.0000x reference)
"""Trainium2 Bass kernel for nn_Basic_Operator_59365037965641 (v2).

out = w0*(x+y) + w1*x*y + w2*x/(|y|+eps) + w3*y/(|x|+eps)
    + w4*x*sin(y) + w5*y*sin(x),   w = softmax(param,0).sum(1)

Design (cost-model-driven; all engines balanced near the DMA roofline):
  - x/y uploaded as bf16 (halves inbound DMA, unlocks DVE 2x TensorTensor).
  - One fused custom DVE op per divide term:
        q = Src1 * recip1nr(|Src0| + eps)
    -|x| via BITWISE_OR with -0.0 (then eps - (-|x|)), seed via the
    BITWISE_NOT exponent-flip trick, one Newton step against the constant
    2.0 hoisted as a One+One latch. 8 ALU stages -- exactly fits the v3
    DVE pipeline. f32r output, emitted in PSUB-sized halves so buffers
    recycle at subtile granularity.
  - ACT: sin(x), sin(y) (bf16 in -> bf16 out), no range wrap (HW-validated).
  - DVE/Pool: m1=x*y, m2=x*sin(y), m3=y*sin(x) bf16 tensor_tensor,
    column-split between the engines (KPOOLC cols to Pool).
  - PE: psO = w0*x + w0*y + w1*m1 + w4*m2 + w5*m3 (bf16 diags)
             + w2*q2 + w3*q1 (f32r diags, exact weights)
    term-major per PSUB subtile (one ldweights per term).
  - ACT evacuates PSUM to bf16, DMA out bf16 (halved outbound traffic).

Data-parallel across 8 cores on the leading dim (flattened rows).
"""

import os
import sys

import numpy as np

sys.path.insert(0, "/opt/trn_rl_repo")

from contextlib import ExitStack

import concourse.bass as bass
import concourse.tile as tile
from concourse import bacc, mybir

EPS = 1e-8
# seed scale for the 1-Newton reciprocal (optimized for NR const == 2.0)
CSEED = -0.2352941386146546
N_CORES = 8
FULL_ROWS = 16384            # 4*4096
COLS = 4096
SHARD_ROWS = FULL_ROWS // N_CORES       # 2048
P = 128
F_TILE = int(os.environ.get("KFT", "2048"))    # columns per [128, F] elementwise tile
N_TILES = SHARD_ROWS // P                # 16 row blocks
F_CHUNK = 512                            # matmul moving-dim chunk (1 PSUM bank)
PSUB = int(os.environ.get("KPSUB", "2048"))    # psO subtile
POOLC = int(os.environ.get("KPOOLC", "2944"))  # pool cols of the 3*F_TILE TT pot
IOB = int(os.environ.get("KIOB", "3"))
QB = int(os.environ.get("KQB", "3"))
MB = int(os.environ.get("KMB", "2"))
SINB = int(os.environ.get("KSINB", "2"))
OB = int(os.environ.get("KOB", "2"))
ODT = os.environ.get("KODT", "bf16")
QORD = os.environ.get("KQORD", "mid")   # late|mid: q terms position in PE order

f32 = mybir.dt.float32
f32r = mybir.dt.float32r
bf16 = mybir.dt.bfloat16
Alu = mybir.AluOpType
Act = mybir.ActivationFunctionType

_cached = {}


def _register_fused_recip_mul():
    import concourse.dve_ops as D
    from concourse.dve_ops import DveOp
    from concourse.dve_spec import Src0, Src1, C0, C1, C2, One, Bin, Spec
    from concourse.dve_uop import AluOp

    name = "ABS_RECIP_MUL_ANT"
    if name in D._SUB_OPCODE_FOR_NAME:
        return [o for o in D.OPS if o.name == name][0]

    # |x|+eps without an 0x7fffffff mask (NaN immediates get canonicalized
    # in the const-load path): OR with -0.0 forces the sign bit, giving
    # -|x|, and eps - (-|x|) = |x| + eps. Two stages, plain immediates.
    _negabs = Bin(AluOp.BITWISE_OR, Src0, C0)  # C0 = -0.0 -> -|Src0|
    _b = C1 - _negabs                          # C1 = eps
    _nb = Bin(AluOp.BITWISE_NOT, _b, _b)
    _y0 = _nb * C2                             # C2 = seed scale
    _v = Src1 * _y0
    _t = _b * _y0
    _u = (One + One) - _t                      # const 2.0, hoisted to latch
    body = _v * _u

    def ref(in0, in1, c0, c1, c2):
        b = np.abs(in0.astype(np.float32)) + np.float32(c1)
        nb = (~b.view(np.int32)).view(np.float32)
        y0 = (nb * np.float32(c2)).astype(np.float32)
        return ((in1.astype(np.float32) * y0) * (np.float32(2.0) - b * y0)).astype(
            np.float32
        )

    op = DveOp(name, Spec(body=body, reference=ref), subdim=False, uops_sha={})
    D.OPS.append(op)
    D._SUB_OPCODE_FOR_NAME[op.name] = D._CUSTOM_DVE_ROW_BASE + len(D.OPS) - 1
    D.CUSTOM_DVE_SPECS[op.name] = op.spec
    import re

    for ver in ("v3", "v4"):
        try:
            op.compile(ver)
        except ValueError as e:
            m = re.search(rf"{ver}: ([0-9a-f]+)", str(e))
            op.uops_sha[ver] = m.group(1)
    op.compile("v3")
    return op


def build_bass():
    op_q = _register_fused_recip_mul()

    nc = bacc.Bacc("TRN2", target_bir_lowering=False, debug=False)

    x_d = nc.dram_tensor("x", [SHARD_ROWS, COLS], bf16, kind="ExternalInput")
    y_d = nc.dram_tensor("y", [SHARD_ROWS, COLS], bf16, kind="ExternalInput")
    # 4 stacked [128,128] bf16 diagonals: w0, w1, w4, w5
    dgb_d = nc.dram_tensor("diags_bf", [P, 4 * P], bf16, kind="ExternalInput")
    # 2 stacked [128,128] f32 diagonals: w2, w3
    dgf_d = nc.dram_tensor("diags_f32", [P, 2 * P], f32, kind="ExternalInput")
    o_dt = bf16 if ODT == "bf16" else f32
    out_d = nc.dram_tensor("out", [SHARD_ROWS, COLS], o_dt, kind="ExternalOutput")

    xv = x_d.ap().rearrange("(n p) c -> n p c", p=P)   # [16, 128, 4096]
    yv = y_d.ap().rearrange("(n p) c -> n p c", p=P)
    ov = out_d.ap().rearrange("(n p) c -> n p c", p=P)
    col_tiles = COLS // F_TILE
    n_sub = F_TILE // PSUB

    with tile.TileContext(nc) as tc, ExitStack() as ctx:
        const_pool = ctx.enter_context(tc.tile_pool(name="const", bufs=1))
        io_pool = ctx.enter_context(tc.tile_pool(name="io", bufs=IOB))
        sin_pool = ctx.enter_context(tc.tile_pool(name="sin", bufs=SINB))
        m_pool = ctx.enter_context(tc.tile_pool(name="m", bufs=MB))
        q_pool = ctx.enter_context(tc.tile_pool(name="q", bufs=QB))
        ps_pool = ctx.enter_context(
            tc.tile_pool(name="ps", bufs=8 // (PSUB // 512), space="PSUM")
        )
        o_pool = ctx.enter_context(tc.tile_pool(name="o", bufs=OB))

        diags_b = const_pool.tile([P, 4 * P], bf16)
        nc.sync.dma_start(diags_b[:], dgb_d.ap())
        diags_f = const_pool.tile([P, 2 * P], f32r)
        nc.sync.dma_start(diags_f[:], dgf_d.ap().bitcast(f32r))
        d_w0 = diags_b[:, 0 * P : 1 * P]
        d_w1 = diags_b[:, 1 * P : 2 * P]
        d_w4 = diags_b[:, 2 * P : 3 * P]
        d_w5 = diags_b[:, 3 * P : 4 * P]
        d_w2 = diags_f[:, 0 * P : 1 * P]
        d_w3 = diags_f[:, 1 * P : 2 * P]

        for r in range(N_TILES):
            for cidx in range(col_tiles):
                csl = slice(cidx * F_TILE, (cidx + 1) * F_TILE)
                x_t = io_pool.tile([P, F_TILE], bf16, tag="x")
                nc.sync.dma_start(x_t[:], xv[r][:, csl])
                y_t = io_pool.tile([P, F_TILE], bf16, tag="y")
                nc.sync.dma_start(y_t[:], yv[r][:, csl])

                # fused divide terms (fp32, one half-tile per subtile so
                # buffers recycle at subtile granularity), DVE custom
                q2h, q1h = [], []
                for s in range(n_sub):
                    ssl = slice(s * PSUB, (s + 1) * PSUB)
                    q2 = q_pool.tile([P, PSUB], f32r, tag=f"q2{s}")
                    nc.vector._custom_dve(
                        op_q, out=q2[:], in0=y_t[:, ssl], in1=x_t[:, ssl],
                        s0=-0.0, s1=EPS, imm2=CSEED,
                    )
                    q2h.append(q2)
                    q1 = q_pool.tile([P, PSUB], f32r, tag=f"q1{s}")
                    nc.vector._custom_dve(
                        op_q, out=q1[:], in0=x_t[:, ssl], in1=y_t[:, ssl],
                        s0=-0.0, s1=EPS, imm2=CSEED,
                    )
                    q1h.append(q1)

                # sins on ACT (bf16 -> bf16)
                s_x = sin_pool.tile([P, F_TILE], bf16, tag="sx")
                s_y = sin_pool.tile([P, F_TILE], bf16, tag="sy")
                nc.scalar.activation(s_x[:], x_t[:], Act.Sin)
                nc.scalar.activation(s_y[:], y_t[:], Act.Sin)

                # bf16 products, column-split DVE/Pool (Pool chunked)
                m1 = m_pool.tile([P, F_TILE], bf16, tag="m1")  # x*y
                m2 = m_pool.tile([P, F_TILE], bf16, tag="m2")  # x*sin(y)
                m3 = m_pool.tile([P, F_TILE], bf16, tag="m3")  # y*sin(x)
                pool_left = POOLC
                for dst, a, b in ((m1, x_t, y_t), (m3, y_t, s_x), (m2, x_t, s_y)):
                    pc = min(pool_left, F_TILE)
                    pool_left -= pc
                    for p0 in range(0, pc, PSUB):
                        p1 = min(p0 + PSUB, pc)
                        nc.gpsimd.tensor_tensor(
                            dst[:, p0:p1], a[:, p0:p1], b[:, p0:p1], Alu.mult
                        )
                    if pc < F_TILE:
                        nc.vector.tensor_tensor(
                            dst[:, pc:], a[:, pc:], b[:, pc:], Alu.mult
                        )

                # PE accumulation (term-major per subtile), ACT evac, DMA out
                for s in range(n_sub):
                    if QORD == "mid":
                        terms = (
                            (d_w0, x_t, None, False),
                            (d_w0, y_t, None, False),
                            (d_w1, m1, None, False),
                            (d_w2, q2h[s], None, True),
                            (d_w3, q1h[s], None, True),
                            (d_w5, m3, None, False),
                            (d_w4, m2, None, False),
                        )
                    else:
                        terms = (
                            (d_w0, x_t, None, False),
                            (d_w0, y_t, None, False),
                            (d_w1, m1, None, False),
                            (d_w5, m3, None, False),
                            (d_w4, m2, None, False),
                            (d_w2, q2h[s], None, True),
                            (d_w3, q1h[s], None, True),
                        )
                    n_terms = len(terms)
                    o_t = o_pool.tile([P, PSUB], o_dt, tag="o")
                    psO = ps_pool.tile([P, PSUB], f32, tag="ps")
                    for ti, (dg, src, cast, is_half) in enumerate(terms):
                        for c in range(PSUB // F_CHUNK):
                            pcs = slice(c * F_CHUNK, (c + 1) * F_CHUNK)
                            cs = pcs if is_half else slice(
                                s * PSUB + c * F_CHUNK, s * PSUB + (c + 1) * F_CHUNK
                            )
                            mv = src[:, cs] if cast is None else src[:, cs].bitcast(cast)
                            nc.tensor.matmul(
                                psO[:, pcs], dg, mv,
                                start=(ti == 0), stop=(ti == n_terms - 1),
                            )
                    nc.scalar.activation(o_t[:], psO[:], Act.Copy, bias=0.0, scale=1.0)
                    nc.sync.dma_start(
                        ov[r][:, cidx * F_TILE + s * PSUB : cidx * F_TILE + (s + 1) * PSUB],
                        o_t[:],
                    )

    nc.finalize()
    return nc


def _get_program():
    if "prog" not in _cached:
        _cached["prog"] = build_bass()
    return _cached["prog"]


def _program_for_timing(param=None):
    return _get_program()


def _weights(param):
    param = np.asarray(param, dtype=np.float64)
    m = param.max(axis=0, keepdims=True)
    e = np.exp(param - m)
    soft = e / e.sum(axis=0, keepdims=True)
    return soft.sum(axis=1)  # [6]


def _run(x, y, param, trace=False):
    import ml_dtypes
    from concourse.bass_utils import run_bass_kernel_spmd

    x = np.asarray(x)
    y = np.asarray(y)
    w = _weights(param)
    nc = _get_program()

    bf = ml_dtypes.bfloat16
    xf = np.ascontiguousarray(x.reshape(FULL_ROWS, COLS)).astype(bf)
    yf = np.ascontiguousarray(y.reshape(FULL_ROWS, COLS)).astype(bf)

    eye = np.eye(P, dtype=np.float32)
    dgb = np.concatenate(
        [eye * np.float32(w[i]) for i in (0, 1, 4, 5)], axis=1
    ).astype(bf)
    dgf = np.concatenate(
        [eye * np.float32(w[i]) for i in (2, 3)], axis=1
    ).astype(np.float32)

    in_maps = []
    for c in range(N_CORES):
        rows = slice(c * SHARD_ROWS, (c + 1) * SHARD_ROWS)
        in_maps.append(
            {
                "x": xf[rows], "y": yf[rows], "diags_bf": dgb,
                "diags_f32": dgf,
            }
        )

    res = run_bass_kernel_spmd(
        nc, in_maps, core_ids=list(range(N_CORES)), trace=trace
    )
    out = np.empty((FULL_ROWS, COLS), dtype=np.float32)
    for c in range(N_CORES):
        out[c * SHARD_ROWS : (c + 1) * SHARD_ROWS] = (
            res.results[c]["out"].astype(np.float32)
        )
    return out.reshape(x.shape), res


def kernel(x, y, param):
    out, _ = _run(x, y, param, trace=False)
    return out


# revision 10
# speedup vs baseline: 1.0188x; 1.0188x over previous
"""Trainium2 Bass kernel for nn_Basic_Operator_59365037965641 (v2).

out = w0*(x+y) + w1*x*y + w2*x/(|y|+eps) + w3*y/(|x|+eps)
    + w4*x*sin(y) + w5*y*sin(x),   w = softmax(param,0).sum(1)

Design (cost-model-driven; all engines balanced near the DMA roofline):
  - x/y uploaded as bf16 (halves inbound DMA, unlocks DVE 2x TensorTensor).
  - One fused custom DVE op per divide term:
        q = Src1 * recip1nr(|Src0| + eps)
    -|x| via BITWISE_OR with -0.0 (then eps - (-|x|)), seed via the
    BITWISE_NOT exponent-flip trick, one Newton step against the constant
    2.0 hoisted as a One+One latch. 8 ALU stages -- exactly fits the v3
    DVE pipeline. f32r output, emitted in PSUB-sized halves so buffers
    recycle at subtile granularity.
  - ACT: sin(x), sin(y) (bf16 in -> bf16 out), no range wrap (HW-validated).
  - DVE/Pool: m1=x*y, m2=x*sin(y), m3=y*sin(x) bf16 tensor_tensor,
    column-split between the engines (KPOOLC cols to Pool).
  - PE: psO = w0*x + w0*y + w1*m1 + w4*m2 + w5*m3 (bf16 diags)
             + w2*q2 + w3*q1 (f32r diags, exact weights)
    term-major per PSUB subtile (one ldweights per term).
  - ACT evacuates PSUM to bf16, DMA out bf16 (halved outbound traffic).

Data-parallel across 8 cores on the leading dim (flattened rows).
"""

import os
import sys

import numpy as np

sys.path.insert(0, "/opt/trn_rl_repo")

from contextlib import ExitStack

import concourse.bass as bass
import concourse.tile as tile
from concourse import bacc, mybir

EPS = 1e-8
# seed scale for the 1-Newton reciprocal (optimized for NR const == 2.0)
CSEED = -0.2352941386146546
N_CORES = 8
FULL_ROWS = 16384            # 4*4096
COLS = 4096
SHARD_ROWS = FULL_ROWS // N_CORES       # 2048
P = 128
F_TILE = int(os.environ.get("KFT", "2048"))    # columns per [128, F] elementwise tile
N_TILES = SHARD_ROWS // P                # 16 row blocks
F_CHUNK = 512                            # matmul moving-dim chunk (1 PSUM bank)
PSUB = int(os.environ.get("KPSUB", "2048"))    # psO subtile
POOLC = int(os.environ.get("KPOOLC", "2944"))  # pool cols of the 3*F_TILE TT pot
IOB = int(os.environ.get("KIOB", "3"))
QB = int(os.environ.get("KQB", "3"))
MB = int(os.environ.get("KMB", "2"))
SINB = int(os.environ.get("KSINB", "2"))
OB = int(os.environ.get("KOB", "2"))
ODT = os.environ.get("KODT", "bf16")
QORD = os.environ.get("KQORD", "mid")   # late|mid: q terms position in PE order

f32 = mybir.dt.float32
f32r = mybir.dt.float32r
bf16 = mybir.dt.bfloat16
Alu = mybir.AluOpType
Act = mybir.ActivationFunctionType

_cached = {}


def _register_fused_recip_mul():
    import concourse.dve_ops as D
    from concourse.dve_ops import DveOp
    from concourse.dve_spec import Src0, Src1, C0, C1, C2, One, Bin, Spec
    from concourse.dve_uop import AluOp

    name = "ABS_RECIP_MUL_ANT"
    if name in D._SUB_OPCODE_FOR_NAME:
        return [o for o in D.OPS if o.name == name][0]

    # |x|+eps without an 0x7fffffff mask (NaN immediates get canonicalized
    # in the const-load path): OR with -0.0 forces the sign bit, giving
    # -|x|, and eps - (-|x|) = |x| + eps. Two stages, plain immediates.
    _negabs = Bin(AluOp.BITWISE_OR, Src0, C0)  # C0 = -0.0 -> -|Src0|
    _b = C1 - _negabs                          # C1 = eps
    _nb = Bin(AluOp.BITWISE_NOT, _b, _b)
    _y0 = _nb * C2                             # C2 = seed scale
    _v = Src1 * _y0
    _t = _b * _y0
    _u = (One + One) - _t                      # const 2.0, hoisted to latch
    body = _v * _u

    def ref(in0, in1, c0, c1, c2):
        b = np.abs(in0.astype(np.float32)) + np.float32(c1)
        nb = (~b.view(np.int32)).view(np.float32)
        y0 = (nb * np.float32(c2)).astype(np.float32)
        return ((in1.astype(np.float32) * y0) * (np.float32(2.0) - b * y0)).astype(
            np.float32
        )

    op = DveOp(name, Spec(body=body, reference=ref), subdim=False, uops_sha={})
    D.OPS.append(op)
    D._SUB_OPCODE_FOR_NAME[op.name] = D._CUSTOM_DVE_ROW_BASE + len(D.OPS) - 1
    D.CUSTOM_DVE_SPECS[op.name] = op.spec
    import re

    for ver in ("v3", "v4"):
        try:
            op.compile(ver)
        except ValueError as e:
            m = re.search(rf"{ver}: ([0-9a-f]+)", str(e))
            op.uops_sha[ver] = m.group(1)
    op.compile("v3")
    return op


def build_bass():
    op_q = _register_fused_recip_mul()

    nc = bacc.Bacc("TRN2", target_bir_lowering=False, debug=False)

    x_d = nc.dram_tensor("x", [SHARD_ROWS, COLS], bf16, kind="ExternalInput")
    y_d = nc.dram_tensor("y", [SHARD_ROWS, COLS], bf16, kind="ExternalInput")
    # 4 stacked [128,128] bf16 diagonals: w0, w1, w4, w5
    dgb_d = nc.dram_tensor("diags_bf", [P, 4 * P], bf16, kind="ExternalInput")
    # 2 stacked [128,128] f32 diagonals: w2, w3
    dgf_d = nc.dram_tensor("diags_f32", [P, 2 * P], f32, kind="ExternalInput")
    o_dt = bf16 if ODT == "bf16" else f32
    out_d = nc.dram_tensor("out", [SHARD_ROWS, COLS], o_dt, kind="ExternalOutput")

    xv = x_d.ap().rearrange("(n p) c -> n p c", p=P)   # [16, 128, 4096]
    yv = y_d.ap().rearrange("(n p) c -> n p c", p=P)
    ov = out_d.ap().rearrange("(n p) c -> n p c", p=P)
    col_tiles = COLS // F_TILE
    n_sub = F_TILE // PSUB

    with tile.TileContext(nc) as tc, ExitStack() as ctx:
        const_pool = ctx.enter_context(tc.tile_pool(name="const", bufs=1))
        io_pool = ctx.enter_context(tc.tile_pool(name="io", bufs=IOB))
        sin_pool = ctx.enter_context(tc.tile_pool(name="sin", bufs=SINB))
        m_pool = ctx.enter_context(tc.tile_pool(name="m", bufs=MB))
        q_pool = ctx.enter_context(tc.tile_pool(name="q", bufs=QB))
        ps_pool = ctx.enter_context(
            tc.tile_pool(name="ps", bufs=8 // (PSUB // 512), space="PSUM")
        )
        o_pool = ctx.enter_context(tc.tile_pool(name="o", bufs=OB))

        diags_b = const_pool.tile([P, 4 * P], bf16)
        nc.sync.dma_start(diags_b[:], dgb_d.ap())
        diags_f = const_pool.tile([P, 2 * P], f32r)
        nc.sync.dma_start(diags_f[:], dgf_d.ap().bitcast(f32r))
        d_w0 = diags_b[:, 0 * P : 1 * P]
        d_w1 = diags_b[:, 1 * P : 2 * P]
        d_w4 = diags_b[:, 2 * P : 3 * P]
        d_w5 = diags_b[:, 3 * P : 4 * P]
        d_w2 = diags_f[:, 0 * P : 1 * P]
        d_w3 = diags_f[:, 1 * P : 2 * P]

        # half-width first/last tiles: shorter pipeline fill and drain
        tiles = []
        for r in range(N_TILES):
            for cidx in range(col_tiles):
                base = cidx * F_TILE
                last = r == N_TILES - 1 and cidx == col_tiles - 1
                if (r == 0 and cidx == 0) or last:
                    h = F_TILE // 2
                    tiles += [(r, base, h), (r, base + h, h)]
                else:
                    tiles += [(r, base, F_TILE)]
        if True:
            for (r, c0, w) in tiles:
                csl = slice(c0, c0 + w)
                x_t = io_pool.tile([P, w], bf16, tag="x")
                nc.sync.dma_start(x_t[:], xv[r][:, csl])
                y_t = io_pool.tile([P, w], bf16, tag="y")
                nc.sync.dma_start(y_t[:], yv[r][:, csl])

                # fused divide terms (fp32, one half-tile per subtile so
                # buffers recycle at subtile granularity), DVE custom
                q2h, q1h = [], []
                for s in range(n_sub):
                    ssl = slice(0, w)
                    q2 = q_pool.tile([P, w], f32r, tag=f"q2{s}")
                    nc.vector._custom_dve(
                        op_q, out=q2[:], in0=y_t[:, ssl], in1=x_t[:, ssl],
                        s0=-0.0, s1=EPS, imm2=CSEED,
                    )
                    q2h.append(q2)
                    q1 = q_pool.tile([P, w], f32r, tag=f"q1{s}")
                    nc.vector._custom_dve(
                        op_q, out=q1[:], in0=x_t[:, ssl], in1=y_t[:, ssl],
                        s0=-0.0, s1=EPS, imm2=CSEED,
                    )
                    q1h.append(q1)

                # sins on ACT (bf16 -> bf16)
                s_x = sin_pool.tile([P, w], bf16, tag="sx")
                s_y = sin_pool.tile([P, w], bf16, tag="sy")
                nc.scalar.activation(s_x[:], x_t[:], Act.Sin)
                nc.scalar.activation(s_y[:], y_t[:], Act.Sin)

                # bf16 products, column-split DVE/Pool (Pool chunked)
                m1 = m_pool.tile([P, w], bf16, tag="m1")  # x*y
                m2 = m_pool.tile([P, w], bf16, tag="m2")  # x*sin(y)
                m3 = m_pool.tile([P, w], bf16, tag="m3")  # y*sin(x)
                pool_left = POOLC * w // F_TILE
                for dst, a, b in ((m1, x_t, y_t), (m3, y_t, s_x), (m2, x_t, s_y)):
                    pc = min(pool_left, w)
                    pool_left -= pc
                    for p0 in range(0, pc, PSUB):
                        p1 = min(p0 + PSUB, pc)
                        nc.gpsimd.tensor_tensor(
                            dst[:, p0:p1], a[:, p0:p1], b[:, p0:p1], Alu.mult
                        )
                    if pc < w:
                        nc.vector.tensor_tensor(
                            dst[:, pc:], a[:, pc:], b[:, pc:], Alu.mult
                        )

                # PE accumulation (term-major per subtile), ACT evac, DMA out
                for s in range(n_sub):
                    if QORD == "mid":
                        terms = (
                            (d_w0, x_t, None, False),
                            (d_w0, y_t, None, False),
                            (d_w1, m1, None, False),
                            (d_w2, q2h[s], None, True),
                            (d_w3, q1h[s], None, True),
                            (d_w5, m3, None, False),
                            (d_w4, m2, None, False),
                        )
                    else:
                        terms = (
                            (d_w0, x_t, None, False),
                            (d_w0, y_t, None, False),
                            (d_w1, m1, None, False),
                            (d_w5, m3, None, False),
                            (d_w4, m2, None, False),
                            (d_w2, q2h[s], None, True),
                            (d_w3, q1h[s], None, True),
                        )
                    n_terms = len(terms)
                    o_t = o_pool.tile([P, w], o_dt, tag="o")
                    psO = ps_pool.tile([P, w], f32, tag="ps")
                    for ti, (dg, src, cast, is_half) in enumerate(terms):
                        for c in range(w // F_CHUNK):
                            pcs = slice(c * F_CHUNK, (c + 1) * F_CHUNK)
                            cs = pcs
                            mv = src[:, cs] if cast is None else src[:, cs].bitcast(cast)
                            nc.tensor.matmul(
                                psO[:, pcs], dg, mv,
                                start=(ti == 0), stop=(ti == n_terms - 1),
                            )
                    nc.scalar.activation(o_t[:], psO[:], Act.Copy, bias=0.0, scale=1.0)
                    nc.sync.dma_start(ov[r][:, c0 : c0 + w], o_t[:])

    nc.finalize()
    return nc


def _get_program():
    if "prog" not in _cached:
        _cached["prog"] = build_bass()
    return _cached["prog"]


def _program_for_timing(param=None):
    return _get_program()


def _weights(param):
    param = np.asarray(param, dtype=np.float64)
    m = param.max(axis=0, keepdims=True)
    e = np.exp(param - m)
    soft = e / e.sum(axis=0, keepdims=True)
    return soft.sum(axis=1)  # [6]


def _run(x, y, param, trace=False):
    import ml_dtypes
    from concourse.bass_utils import run_bass_kernel_spmd

    x = np.asarray(x)
    y = np.asarray(y)
    w = _weights(param)
    nc = _get_program()

    bf = ml_dtypes.bfloat16
    xf = np.ascontiguousarray(x.reshape(FULL_ROWS, COLS)).astype(bf)
    yf = np.ascontiguousarray(y.reshape(FULL_ROWS, COLS)).astype(bf)

    eye = np.eye(P, dtype=np.float32)
    dgb = np.concatenate(
        [eye * np.float32(w[i]) for i in (0, 1, 4, 5)], axis=1
    ).astype(bf)
    dgf = np.concatenate(
        [eye * np.float32(w[i]) for i in (2, 3)], axis=1
    ).astype(np.float32)

    in_maps = []
    for c in range(N_CORES):
        rows = slice(c * SHARD_ROWS, (c + 1) * SHARD_ROWS)
        in_maps.append(
            {
                "x": xf[rows], "y": yf[rows], "diags_bf": dgb,
                "diags_f32": dgf,
            }
        )

    res = run_bass_kernel_spmd(
        nc, in_maps, core_ids=list(range(N_CORES)), trace=trace
    )
    out = np.empty((FULL_ROWS, COLS), dtype=np.float32)
    for c in range(N_CORES):
        out[c * SHARD_ROWS : (c + 1) * SHARD_ROWS] = (
            res.results[c]["out"].astype(np.float32)
        )
    return out.reshape(x.shape), res


def kernel(x, y, param):
    out, _ = _run(x, y, param, trace=False)
    return out
